# revision 20
# baseline (speedup 1.0000x reference)
"""Multi-head attention block (12 heads, N=2048, C=768) on 8 NeuronCores.

Sharding: core i = (batch b = i//2, head-group g = i%2). Each core computes
attention for 6 heads of one batch plus its slice of the output projection
(row-sharded Wproj); the host sums the two head-group partials per batch.

Per-core dataflow (all matmuls bf16):
  xT [768,2048] bf16 arrives host-transposed; QT/KT [384,2048] bf16 are
  column-major (head h lives at partitions (h%2)*64..+64 of tile h//2).
  V2 is token-major, 128-col stride per head (full 128-col stationary ->
  FWL). Even heads: dims at cols 0-63, ones at col 64 (denominator row
  lands at PSUM partition 64). Odd heads: ones at col 63, dims at cols
  64-127, so U_b lands directly at partitions 64-127 of PSUM and the
  normalized result writes UT[64:128] with no cross-partition DMA hop.

  Heads are processed in pairs (a=2j at PE rows 0-63, b=2j+1 at 64-127).
  Per (pair, 512-query chunk qs, key block k):
    S^T_a -> pss[:, 0:512], S^T_b -> pss[:, 512:1024]  (two matmuls in
      disjoint PE row groups sharing one PSUM tile -> concurrent)
    es = exp(S/8) in ONE instruction for both heads: ACT exp for 10 of 16
      k-blocks; DVE Schraudolph bit-trick exp for the other 6 (es_bits =
      int16(S*A + B) bitcast bf16; A,B from the `expc` input).
    U'_a += V2_a[k]^T @ es[:, 0:512], U'_b += V2_b[k]^T @ es[:, 512:1024]
      (PSUM accumulated over k; denominators via the ones column;
      software-pipelined 4-5 k-steps behind the scores)
  Normalization per head: evacuate U'+denom to SBUF, reciprocal directly
  on the [1,512] denom row, gpsimd partition-broadcast, fused multiply
  into UT -- no DMA round trips.
  out = UT^T-chunks @ Wproj_rows (bf16, PSUM-accumulated). Projection for
  query chunk qs runs as hook pieces inside the next pair; the final
  chunk's projection is software-pipelined (m=0,2 matmuls overlap the last
  pair's normalization, m=1 closes).

Startup: input DMA issue is split across three queues (Sync HWDGE: xT,
wv, wp; Scalar HWDGE: wk, wq; gpsimd SWDGE: bqk/expc/bv) because each
dma_start costs ~0.6us of issue time on its engine. All 16 V tiles and
the remaining QKV m-tiles drip into pair (0,0)'s hook.
"""

import numpy as np
import ml_dtypes
from contextlib import ExitStack

import concourse.bass as bass
import concourse.tile as tile
from concourse import bacc, mybir
from concourse.bass_utils import run_bass_kernel_spmd

N_CORES = 8
C = 768          # model dim
HG = 6           # heads per core
D = 64           # head dim
CHG = HG * D     # 384, per-group qkv width
CC = C // 128    # 6 contraction chunks
MT = CHG // 128  # 3 m-tiles for QT/KT
SCALE = 1.0 / 8.0

# Schraudolph fast-exp constants (bf16 bitcast):
#   es_bits = int16(S * EXP_A + EXP_B); bits reinterpreted as bf16
EXP_A = 128.0 * np.log2(np.e) * SCALE
EXP_B = 16248.72

F32 = mybir.dt.float32
BF16 = mybir.dt.bfloat16
I16 = mybir.dt.int16

BF = ml_dtypes.bfloat16


def build(n_tok: int = 2048):
    NT = n_tok
    KB = NT // 128           # key blocks
    NQ = NT // 512           # 512-wide query chunks
    EXPF = mybir.ActivationFunctionType.Exp

    nc = bacc.Bacc("TRN2", target_bir_lowering=False, debug=False,
                   num_devices=N_CORES)

    xT = nc.dram_tensor("xT", [C, NT], BF16, kind="ExternalInput").ap()
    wq = nc.dram_tensor("wq", [C, CHG], BF16, kind="ExternalInput").ap()
    wk = nc.dram_tensor("wk", [C, CHG], BF16, kind="ExternalInput").ap()
    wv = nc.dram_tensor("wv", [C, CHG], BF16, kind="ExternalInput").ap()
    wp = nc.dram_tensor("wp", [CHG, C], BF16, kind="ExternalInput").ap()
    bqk = nc.dram_tensor("bqk", [128, 2 * MT], F32, kind="ExternalInput").ap()
    bv = nc.dram_tensor("bv", [1, CHG], F32, kind="ExternalInput").ap()
    expc = nc.dram_tensor("expc", [128, 2], F32, kind="ExternalInput").ap()
    out = nc.dram_tensor("out", [NT, C], BF16, kind="ExternalOutput").ap()

    with tile.TileContext(nc) as tc, ExitStack() as ctx:
        wpool = ctx.enter_context(tc.tile_pool(name="w", bufs=1))
        perm = ctx.enter_context(tc.tile_pool(name="perm", bufs=1))
        # PSUM budget (8 banks): "ps" 3 x [128,1024] (6 banks) shared by
        # scores pipeline / v tiles / projection; "psu" 2 x [128,512]
        # (2 banks) holds the attnV accumulators of the in-flight pair.
        psum = ctx.enter_context(tc.tile_pool(name="ps", bufs=3, space="PSUM"))
        psum_u = ctx.enter_context(tc.tile_pool(name="psu", bufs=2,
                                                space="PSUM"))

        # ---- persistent SBUF ----
        wq_t = [wpool.tile([128, CHG], BF16, tag=f"wq{c}", name=f"wq{c}")
                for c in range(CC)]
        wk_t = [wpool.tile([128, CHG], BF16, tag=f"wk{c}", name=f"wk{c}")
                for c in range(CC)]
        wv_t = [wpool.tile([128, CHG], BF16, tag=f"wv{c}", name=f"wv{c}")
                for c in range(CC)]
        wp_t = [wpool.tile([128, C], BF16, tag=f"wp{m}", name=f"wp{m}")
                for m in range(MT)]
        bqk_t = wpool.tile([128, 2 * MT], F32, tag="bqk")
        bv_row = wpool.tile([1, CHG], F32, tag="bvr")
        bv_bc = wpool.tile([128, CHG], F32, tag="bvb")
        expc_t = wpool.tile([128, 2], F32, tag="expc")

        QT = [perm.tile([128, NT], BF16, tag=f"qt{m}", name=f"qtt{m}")
              for m in range(MT)]
        KT = [perm.tile([128, NT], BF16, tag=f"kt{m}", name=f"ktt{m}")
              for m in range(MT)]
        V2 = [perm.tile([128, HG, 128], BF16, tag=f"v2{t}", name=f"v2t{t}")
              for t in range(KB)]
        UT = [perm.tile([128, NT], BF16, tag=f"ut{m}", name=f"utt{m}")
              for m in range(MT)]

        spool = ctx.enter_context(tc.tile_pool(name="es", bufs=14))
        rpool = ctx.enter_context(tc.tile_pool(name="rb", bufs=3))
        opool = ctx.enter_context(tc.tile_pool(name="ost", bufs=3))
        xpool = ctx.enter_context(tc.tile_pool(name="xt", bufs=1))

        xt = [xpool.tile([128, NT], BF16, tag=f"x{c}", name=f"xt{c}")
              for c in range(CC)]

        # ---- input DMA, split across three issuing engines ----
        # gpsimd software DGE: the tiny side inputs
        nc.gpsimd.dma_start(bqk_t[:], bqk)
        nc.gpsimd.dma_start(expc_t[:], expc)
        nc.gpsimd.dma_start(bv_row[0:1, :], bv[0:1, :])
        # Scalar HWDGE: wk then wq (prologue order: k m-tile 0 first)
        for c in range(CC):
            nc.scalar.dma_start(wk_t[c][:], wk[c * 128:(c + 1) * 128, :])
        for c in range(CC):
            nc.scalar.dma_start(wq_t[c][:], wq[c * 128:(c + 1) * 128, :])
        # Sync HWDGE: xT (needed by everything), wv, wp
        for c in range(CC):
            nc.sync.dma_start(xt[c][:], xT[c * 128:(c + 1) * 128, :])
        for c in range(CC):
            nc.sync.dma_start(wv_t[c][:], wv[c * 128:(c + 1) * 128, :])
        for m in range(MT):
            nc.sync.dma_start(wp_t[m][:], wp[m * 128:(m + 1) * 128, :])

        nc.gpsimd.partition_broadcast(bv_bc[:], bv_row[0:1, :])
        # V2 constants: even head dims 0:64 / ones col 64; odd head ones
        # col 32 / dims 64:128 (denominator must land on a 32-aligned
        # PSUM partition). Flattened per-pair view [128, 3, 256]: ones at
        # cols {64, 160}, zeros at 65..159 and 161..191.
        bv3 = bv_bc[:, 0:3].rearrange("p (a b) -> p a b", a=MT)
        for t in range(KB):
            v2p = V2[t].rearrange("p (a b) c -> p a (b c)", b=2)
            nc.gpsimd.tensor_scalar(
                v2p[:, :, 64:65], bv3,
                0.0, 1.0, mybir.AluOpType.mult, mybir.AluOpType.add)
            nc.gpsimd.tensor_scalar(
                v2p[:, :, 128:129], bv3,
                0.0, 1.0, mybir.AluOpType.mult, mybir.AluOpType.add)
            nc.gpsimd.memset(v2p[:, :, 65:128], 0)
            nc.gpsimd.memset(v2p[:, :, 129:192], 0)

        def qk_group(m, n, which):
            wt, dst, bcol = ((wq_t, QT, m) if which == "q"
                             else (wk_t, KT, MT + m))
            ps = psum.tile([128, 512], F32, tag="ps",
                           name=f"psqk{which}{m}_{n}")
            for c in range(CC):
                nc.tensor.matmul(
                    ps[:], wt[c][:, m * 128:(m + 1) * 128],
                    xt[c][:, n * 512:(n + 1) * 512],
                    start=(c == 0), stop=(c == CC - 1))
            nc.vector.tensor_scalar_add(
                dst[m][:, n * 512:(n + 1) * 512], ps[:],
                bqk_t[:, bcol:bcol + 1])

        def v_tile(t):
            pv = psum.tile([128, 1024], F32, tag="ps", name=f"psv{t}")
            ps = pv[:, 0:CHG]
            for c in range(CC):
                nc.tensor.matmul(ps, xt[c][:, t * 128:(t + 1) * 128],
                                 wv_t[c][:],
                                 start=(c == 0), stop=(c == CC - 1))
            psr = ps.rearrange("p (a b d) -> p a b d", a=MT, b=2)
            bvr = bv_bc[:].rearrange("p (a b d) -> p a b d", a=MT, b=2)
            v2p = V2[t].rearrange("p (a b) c -> p a (b c)", b=2)
            nc.vector.tensor_add(v2p[:, :, 0:64], psr[:, :, 0, :],
                                 bvr[:, :, 0, :])
            nc.vector.tensor_add(v2p[:, :, 192:256], psr[:, :, 1, :],
                                 bvr[:, :, 1, :])

        # ---- attention pieces ----
        def attn_pair(qs, j, k_hook=None, hook_first=True):
            ha, hb = 2 * j, 2 * j + 1
            q0 = qs * 512
            psu = [psum_u.tile([128, 512], F32, tag="psu",
                               name=f"psu{j}_{qs}_{i}") for i in range(2)]

            def emit_pv(k, es):
                for i, h in enumerate((ha, hb)):
                    nc.tensor.matmul(
                        psu[i][:, :], V2[k][:, h, :],
                        es[:, i * 512:(i + 1) * 512],
                        start=(k == 0), stop=(k == KB - 1))

            def emit_scores(k):
                pss = psum.tile([128, 1024], F32, tag="ps",
                                name=f"pss{j}_{qs}_{k}")
                for i, off in ((0, 0), (1, 64)):
                    nc.tensor.matmul(
                        pss[:, i * 512:(i + 1) * 512],
                        KT[j][off:off + 64, k * 128:(k + 1) * 128],
                        QT[j][off:off + 64, q0:q0 + 512],
                        start=True, stop=True)
                es = spool.tile([128, 1024], BF16, tag="es",
                                name=f"es{j}_{qs}_{k}")
                if k % 2 == 1 and k != KB - 1:
                    nc.vector.tensor_scalar(
                        es[:].bitcast(I16), pss[:],
                        expc_t[:, 0:1], expc_t[:, 1:2],
                        mybir.AluOpType.mult, mybir.AluOpType.add)
                else:
                    nc.scalar.activation(es[:], pss[:], EXPF, scale=SCALE)
                return es

            # software pipeline: scores/exp run 4-5 k-steps ahead of the
            # attnV matmuls; hook pieces (QKV m-tiles / V tiles / proj)
            # fill the tensor slack left by exp latency.
            esq = []
            next_pv = 0
            for kk in range(0, KB, 2):
                if k_hook is not None and hook_first:
                    k_hook(kk)
                for k in range(kk, min(kk + 2, KB)):
                    esq.append(emit_scores(k))
                if k_hook is not None and not hook_first:
                    k_hook(kk)
                while next_pv < kk - 4:
                    emit_pv(next_pv, esq[next_pv])
                    next_pv += 1
            while next_pv < KB:
                emit_pv(next_pv, esq[next_pv])
                next_pv += 1

            # Normalization. partition_broadcast reads its source via the
            # Q7 cpu0 read port, which only works from partition 0 — so
            # every broadcast source sits at partition 0 of its tile.
            # head b first (its denom row 0 needs no hop; frees psu
            # earlier for the DMA-lagged head a chain to catch up).
            urb = rpool.tile([128, 512], F32, tag="ur2",
                             name=f"urb{j}_{qs}")
            nc.vector.tensor_copy(urb[64:128, :], psu[1][64:128, :])
            nc.scalar.copy(urb[0:1, :], psu[1][0:1, :])
            rcb = rpool.tile([128, 512], F32, tag="rb2",
                             name=f"rcb{j}_{qs}")
            nc.vector.reciprocal(rcb[0:1, :], urb[0:1, :])
            nc.gpsimd.partition_broadcast(rcb[0:128, :], rcb[0:1, :])
            nc.vector.tensor_mul(UT[j][64:128, q0:q0 + 512],
                                 urb[64:128, :], rcb[64:128, :])
            # head a: U rows 0-63, denom row 64 (psu[0]); one-row DMA hop
            # brings the reciprocal to partition 0 for the broadcast.
            ura = rpool.tile([128, 512], F32, tag="ur",
                             name=f"ura{j}_{qs}")
            nc.scalar.copy(ura[0:65, :], psu[0][0:65, :])
            rca = rpool.tile([128, 512], F32, tag="rb",
                             name=f"rca{j}_{qs}")
            nc.vector.reciprocal(rca[64:65, :], ura[64:65, :])
            nc.sync.dma_start(rca[0:1, :], rca[64:65, :])
            nc.gpsimd.partition_broadcast(rca[0:64, :], rca[0:1, :])
            nc.vector.tensor_mul(UT[j][0:64, q0:q0 + 512],
                                 ura[0:64, :], rca[0:64, :])

        def proj_qb(qb):
            pt = psum.tile([128, 1024], F32, tag="ps", name=f"pt{qb}")
            for m in range(MT):
                lhsT = UT[m][:, qb * 128:(qb + 1) * 128]
                nc.tensor.matmul(pt[:, 0:512], lhsT, wp_t[m][:, 0:512],
                                 start=(m == 0), stop=(m == MT - 1))
                nc.tensor.matmul(pt[:, 512:768], lhsT,
                                 wp_t[m][:, 512:768],
                                 start=(m == 0), stop=(m == MT - 1))
            ot = opool.tile([128, C], BF16, tag="ost", name=f"ot{qb}")
            nc.scalar.copy(ot[:, 0:512], pt[:, 0:512])
            nc.vector.tensor_copy(ot[:, 512:768], pt[:, 512:768])
            nc.sync.dma_start(out[qb * 128:(qb + 1) * 128, :], ot[:])

        # Final-chunk projection, software-pipelined: m=0 and m=2 partial
        # sums run while pair (3,1) normalizes; m=1 closes each psum tile.
        _tail_pts = {}

        def tail_open(qb):
            pt = psum.tile([128, 1024], F32, tag="ps", name=f"pt{qb}")
            _tail_pts[qb] = pt
            for m in (0, 2):
                lhsT = UT[m][:, qb * 128:(qb + 1) * 128]
                nc.tensor.matmul(pt[:, 0:512], lhsT, wp_t[m][:, 0:512],
                                 start=(m == 0), stop=False)
                nc.tensor.matmul(pt[:, 512:768], lhsT,
                                 wp_t[m][:, 512:768],
                                 start=(m == 0), stop=False)

        def tail_close(qb):
            pt = _tail_pts[qb]
            lhsT = UT[1][:, qb * 128:(qb + 1) * 128]
            nc.tensor.matmul(pt[:, 0:512], lhsT, wp_t[1][:, 0:512],
                             start=False, stop=True)
            nc.tensor.matmul(pt[:, 512:768], lhsT, wp_t[1][:, 512:768],
                             start=False, stop=True)
            ot = opool.tile([128, C], BF16, tag="ost", name=f"ot{qb}")
            nc.scalar.copy(ot[:, 0:512], pt[:, 0:512])
            nc.vector.tensor_copy(ot[:, 512:768], pt[:, 512:768])
            nc.sync.dma_start(out[qb * 128:(qb + 1) * 128, :], ot[:])

        # ---- emission schedule ----
        # Prologue: only K/Q m-tile 0 n-chunk 0. Everything else (all 16 V
        # tiles, remaining K/Q m-tiles and n-chunks, projections) drips
        # into the attention loop via hook pieces.
        pieces = []
        for t in range(3):
            pieces.append(lambda t=t: v_tile(t))
        pieces.append(lambda: qk_group(0, 1, "k"))
        pieces.append(lambda: v_tile(3))
        pieces.append(lambda: v_tile(4))
        pieces.append(lambda: qk_group(0, 2, "k"))
        pieces.append(lambda: v_tile(5))
        pieces.append(lambda: v_tile(6))
        pieces.append(lambda: qk_group(0, 3, "k"))
        for t in range(7, KB):
            pieces.append(lambda t=t: v_tile(t))
        # pair (0,1) prerequisites, then (0,2), then qs>=1 Q chunks
        pieces.append(lambda: qk_group(1, 0, "q"))
        for n in range(NQ):
            pieces.append(lambda n=n: qk_group(1, n, "k"))
        pieces.append(lambda: qk_group(2, 0, "q"))
        for n in range(NQ):
            pieces.append(lambda n=n: qk_group(2, n, "k"))
        for n in range(1, NQ):
            for m in range(MT):
                pieces.append(lambda m=m, n=n: qk_group(m, n, "q"))

        def hook(kk):
            n = 2 if len(pieces) > 4 else 1
            for _ in range(n):
                if pieces:
                    pieces.pop(0)()

        def hook00(kk):
            n = 3 if kk <= 4 else 2
            for _ in range(n):
                if pieces:
                    pieces.pop(0)()

        qk_group(0, 0, "k")
        qk_group(0, 0, "q")
        attn_pair(0, 0, k_hook=hook00, hook_first=False)
        attn_pair(0, 1, k_hook=hook)
        attn_pair(0, 2, k_hook=hook)
        for qb in range(4):
            pieces.append(lambda qb=qb: proj_qb(qb))
        for qs in range(1, NQ - 1):
            attn_pair(qs, 0, k_hook=hook)
            attn_pair(qs, 1, k_hook=hook)
            attn_pair(qs, 2, k_hook=hook)
            for qb in range(qs * 4, qs * 4 + 4):
                pieces.append(lambda qb=qb: proj_qb(qb))
        qs = NQ - 1
        attn_pair(qs, 0, k_hook=hook)
        attn_pair(qs, 2, k_hook=hook)
        attn_pair(qs, 1, k_hook=hook)
        qb0 = qs * 4
        tail_open(qb0)
        tail_open(qb0 + 1)
        tail_open(qb0 + 2)
        tail_close(qb0)
        tail_open(qb0 + 3)
        tail_close(qb0 + 1)
        tail_close(qb0 + 2)
        tail_close(qb0 + 3)

    nc.compile()
    return nc


_built = {}


def _get_nc(n_tok=2048):
    if n_tok not in _built:
        _built[n_tok] = build(n_tok)
    return _built[n_tok]


def make_in_maps(x, Wqkv, bqkv, Wproj, exp_b=EXP_B):
    B, NT, _ = x.shape
    x = np.ascontiguousarray(np.asarray(x, dtype=np.float32))
    Wqkv = np.asarray(Wqkv, dtype=np.float32)
    bqkv = np.asarray(bqkv, dtype=np.float32)
    Wproj = np.asarray(Wproj, dtype=np.float32)
    expc = np.zeros((128, 2), dtype=np.float32)
    expc[:, 0] = EXP_A
    expc[:, 1] = exp_b
    in_maps = []
    for i in range(N_CORES):
        b, g = i // 2, i % 2
        s = g * CHG
        bq = bqkv[s:s + CHG].reshape(MT, 128).T
        bk = bqkv[C + s:C + s + CHG].reshape(MT, 128).T
        in_maps.append({
            "xT": np.ascontiguousarray(x[b].T.astype(BF)),
            "wq": np.ascontiguousarray(Wqkv[:, s:s + CHG].astype(BF)),
            "wk": np.ascontiguousarray(Wqkv[:, C + s:C + s + CHG].astype(BF)),
            "wv": np.ascontiguousarray(
                Wqkv[:, 2 * C + s:2 * C + s + CHG].astype(BF)),
            "wp": np.ascontiguousarray(Wproj[s:s + CHG, :].astype(BF)),
            "bqk": np.ascontiguousarray(
                np.concatenate([bq, bk], axis=1)).astype(np.float32),
            "bv": np.ascontiguousarray(
                bqkv[2 * C + s:2 * C + s + CHG][None, :]).astype(np.float32),
            "expc": expc,
        })
    return in_maps


def gather(results, bproj, B, NT):
    parts = [np.asarray(results[i]["out"], dtype=np.float32)
             for i in range(N_CORES)]
    out = np.stack([parts[2 * b] + parts[2 * b + 1] for b in range(B)])
    return (out + np.asarray(bproj, np.float32)[None, None, :]).astype(np.float32)


def kernel(x, Wqkv, bqkv, Wproj, bproj, _trace=False, _exp_b=EXP_B):
    x = np.asarray(x)
    B, NT, _ = x.shape
    nc = _get_nc(NT)
    in_maps = make_in_maps(x, Wqkv, bqkv, Wproj, exp_b=_exp_b)
    res = run_bass_kernel_spmd(nc, in_maps, core_ids=list(range(N_CORES)),
                               trace=_trace)
    out = gather(res.results, bproj, B, NT)
    if _trace:
        return out, res
    return out


# revision 21
# speedup vs baseline: 1.5310x; 1.5310x over previous
"""Multi-head attention block (12 heads, N=2048, C=768) on 8 NeuronCores.

Sharding: core i = (batch b = i//2, head-group g = i%2). Each core computes
attention for 6 heads of one batch plus its slice of the output projection
(row-sharded Wproj); the host sums the two head-group partials per batch.

Per-core dataflow (all matmuls bf16):
  xT [768,2048] bf16 arrives host-transposed; QT/KT [384,2048] bf16 are
  column-major (head h lives at partitions (h%2)*64..+64 of tile h//2).
  V2 is token-major, 128-col stride per head (full 128-col stationary ->
  FWL). Even heads: dims at cols 0-63, ones at col 64 (denominator row
  lands at PSUM partition 64). Odd heads: ones at col 63, dims at cols
  64-127, so U_b lands directly at partitions 64-127 of PSUM and the
  normalized result writes UT[64:128] with no cross-partition DMA hop.

  Heads are processed in pairs (a=2j at PE rows 0-63, b=2j+1 at 64-127).
  Per (pair, 512-query chunk qs, key block k):
    S^T_a -> pss[:, 0:512], S^T_b -> pss[:, 512:1024]  (two matmuls in
      disjoint PE row groups sharing one PSUM tile -> concurrent)
    es = exp(S/8) in ONE instruction for both heads: ACT exp for 10 of 16
      k-blocks; DVE Schraudolph bit-trick exp for the other 6 (es_bits =
      int16(S*A + B) bitcast bf16; A,B from the `expc` input).
    U'_a += V2_a[k]^T @ es[:, 0:512], U'_b += V2_b[k]^T @ es[:, 512:1024]
      (PSUM accumulated over k; denominators via the ones column;
      software-pipelined 4-5 k-steps behind the scores)
  Normalization per head: evacuate U'+denom to SBUF, reciprocal directly
  on the [1,512] denom row, gpsimd partition-broadcast, fused multiply
  into UT -- no DMA round trips.
  out = UT^T-chunks @ Wproj_rows (bf16, PSUM-accumulated). Projection for
  query chunk qs runs as hook pieces inside the next pair; the final
  chunk's projection is software-pipelined (m=0,2 matmuls overlap the last
  pair's normalization, m=1 closes).

Startup: input DMA issue is split across three queues (Sync HWDGE: xT,
wv, wp; Scalar HWDGE: wk, wq; gpsimd SWDGE: bqk/expc/bv) because each
dma_start costs ~0.6us of issue time on its engine. All 16 V tiles and
the remaining QKV m-tiles drip into pair (0,0)'s hook.
"""

import numpy as np
import ml_dtypes
from contextlib import ExitStack

import concourse.bass as bass
import concourse.tile as tile
from concourse import bacc, mybir
from concourse.bass_utils import run_bass_kernel_spmd

N_CORES = 8
C = 768          # model dim
HG = 6           # heads per core
D = 64           # head dim
CHG = HG * D     # 384, per-group qkv width
CC = C // 128    # 6 contraction chunks
MT = CHG // 128  # 3 m-tiles for QT/KT
SCALE = 1.0 / 8.0

# Schraudolph fast-exp constants (bf16 bitcast):
#   es_bits = int16(S * EXP_A + EXP_B); bits reinterpreted as bf16
EXP_A = 128.0 * np.log2(np.e) * SCALE
EXP_B = 16248.72

F32 = mybir.dt.float32
BF16 = mybir.dt.bfloat16
I16 = mybir.dt.int16

BF = ml_dtypes.bfloat16


def build(n_tok: int = 2048):
    NT = n_tok
    KB = NT // 128           # key blocks
    NQ = NT // 512           # 512-wide query chunks
    EXPF = mybir.ActivationFunctionType.Exp

    nc = bacc.Bacc("TRN2", target_bir_lowering=False, debug=False,
                   num_devices=N_CORES)

    xT = nc.dram_tensor("xT", [C, NT], BF16, kind="ExternalInput").ap()
    wq = nc.dram_tensor("wq", [C, CHG], BF16, kind="ExternalInput").ap()
    wk = nc.dram_tensor("wk", [C, CHG], BF16, kind="ExternalInput").ap()
    wv = nc.dram_tensor("wv", [C, CHG], BF16, kind="ExternalInput").ap()
    wp = nc.dram_tensor("wp", [CHG, C], BF16, kind="ExternalInput").ap()
    bqk = nc.dram_tensor("bqk", [128, 2 * MT], F32, kind="ExternalInput").ap()
    bv = nc.dram_tensor("bv", [1, CHG], F32, kind="ExternalInput").ap()
    expc = nc.dram_tensor("expc", [128, 2], F32, kind="ExternalInput").ap()
    out = nc.dram_tensor("out", [NT, C], BF16, kind="ExternalOutput").ap()

    with tile.TileContext(nc) as tc, ExitStack() as ctx:
        wpool = ctx.enter_context(tc.tile_pool(name="w", bufs=1))
        perm = ctx.enter_context(tc.tile_pool(name="perm", bufs=1))
        # PSUM budget (8 banks): "ps" 3 x [128,1024] (6 banks) shared by
        # scores pipeline / v tiles / projection; "psu" 2 x [128,512]
        # (2 banks) holds the attnV accumulators of the in-flight pair.
        psum = ctx.enter_context(tc.tile_pool(name="ps", bufs=3, space="PSUM"))
        psum_u = ctx.enter_context(tc.tile_pool(name="psu", bufs=2,
                                                space="PSUM"))

        # ---- persistent SBUF ----
        wq_t = [wpool.tile([128, CHG], BF16, tag=f"wq{c}", name=f"wq{c}")
                for c in range(CC)]
        wk_t = [wpool.tile([128, CHG], BF16, tag=f"wk{c}", name=f"wk{c}")
                for c in range(CC)]
        wv_t = [wpool.tile([128, CHG], BF16, tag=f"wv{c}", name=f"wv{c}")
                for c in range(CC)]
        wp_t = [wpool.tile([128, C], BF16, tag=f"wp{m}", name=f"wp{m}")
                for m in range(MT)]
        bqk_t = wpool.tile([128, 2 * MT], F32, tag="bqk")
        bv_row = wpool.tile([1, CHG], F32, tag="bvr")
        bv_bc = wpool.tile([128, CHG], F32, tag="bvb")
        expc_t = wpool.tile([128, 2], F32, tag="expc")

        QT = [perm.tile([128, NT], BF16, tag=f"qt{m}", name=f"qtt{m}")
              for m in range(MT)]
        KT = [perm.tile([128, NT], BF16, tag=f"kt{m}", name=f"ktt{m}")
              for m in range(MT)]
        V2 = [perm.tile([128, HG, 128], BF16, tag=f"v2{t}", name=f"v2t{t}")
              for t in range(KB)]
        UT = [perm.tile([128, NT], BF16, tag=f"ut{m}", name=f"utt{m}")
              for m in range(MT)]

        spool = ctx.enter_context(tc.tile_pool(name="es", bufs=14))
        rpool = ctx.enter_context(tc.tile_pool(name="rb", bufs=3))
        opool = ctx.enter_context(tc.tile_pool(name="ost", bufs=3))
        xpool = ctx.enter_context(tc.tile_pool(name="xt", bufs=1))

        xt = [xpool.tile([128, NT], BF16, tag=f"x{c}", name=f"xt{c}")
              for c in range(CC)]

        # ---- input DMA, split across three issuing engines ----
        # gpsimd software DGE: the tiny side inputs
        nc.gpsimd.dma_start(bqk_t[:], bqk)
        nc.gpsimd.dma_start(expc_t[:], expc)
        nc.gpsimd.dma_start(bv_row[0:1, :], bv[0:1, :])
        # Scalar HWDGE: wk then wq (prologue order: k m-tile 0 first)
        for c in range(CC):
            nc.scalar.dma_start(wk_t[c][:], wk[c * 128:(c + 1) * 128, :])
        for c in range(CC):
            nc.scalar.dma_start(wq_t[c][:], wq[c * 128:(c + 1) * 128, :])
        # Sync HWDGE: xT (needed by everything), wv, wp
        for c in range(CC):
            nc.sync.dma_start(xt[c][:], xT[c * 128:(c + 1) * 128, :])
        for c in range(CC):
            nc.sync.dma_start(wv_t[c][:], wv[c * 128:(c + 1) * 128, :])
        for m in range(MT):
            nc.sync.dma_start(wp_t[m][:], wp[m * 128:(m + 1) * 128, :])

        nc.gpsimd.partition_broadcast(bv_bc[:], bv_row[0:1, :])
        # V2 constants: even head dims 0:64 / ones col 64; odd head ones
        # col 32 / dims 64:128 (denominator must land on a 32-aligned
        # PSUM partition). Flattened per-pair view [128, 3, 256]: ones at
        # cols {64, 160}, zeros at 65..159 and 161..191.
        bv3 = bv_bc[:, 0:3].rearrange("p (a b) -> p a b", a=MT)
        for t in range(KB):
            v2p = V2[t].rearrange("p (a b) c -> p a (b c)", b=2)
            nc.gpsimd.tensor_scalar(
                v2p[:, :, 64:65], bv3,
                0.0, 1.0, mybir.AluOpType.mult, mybir.AluOpType.add)
            nc.gpsimd.tensor_scalar(
                v2p[:, :, 128:129], bv3,
                0.0, 1.0, mybir.AluOpType.mult, mybir.AluOpType.add)
            nc.gpsimd.memset(v2p[:, :, 65:128], 0)
            nc.gpsimd.memset(v2p[:, :, 129:192], 0)

        def qk_group(m, n, which):
            wt, dst, bcol = ((wq_t, QT, m) if which == "q"
                             else (wk_t, KT, MT + m))
            ps = psum.tile([128, 512], F32, tag="ps",
                           name=f"psqk{which}{m}_{n}")
            for c in range(CC):
                nc.tensor.matmul(
                    ps[:], wt[c][:, m * 128:(m + 1) * 128],
                    xt[c][:, n * 512:(n + 1) * 512],
                    start=(c == 0), stop=(c == CC - 1))
            nc.vector.tensor_scalar_add(
                dst[m][:, n * 512:(n + 1) * 512], ps[:],
                bqk_t[:, bcol:bcol + 1])

        def v_tile(t):
            pv = psum.tile([128, 1024], F32, tag="ps", name=f"psv{t}")
            ps = pv[:, 0:CHG]
            for c in range(CC):
                nc.tensor.matmul(ps, xt[c][:, t * 128:(t + 1) * 128],
                                 wv_t[c][:],
                                 start=(c == 0), stop=(c == CC - 1))
            psr = ps.rearrange("p (a b d) -> p a b d", a=MT, b=2)
            bvr = bv_bc[:].rearrange("p (a b d) -> p a b d", a=MT, b=2)
            v2p = V2[t].rearrange("p (a b) c -> p a (b c)", b=2)
            nc.vector.tensor_add(v2p[:, :, 0:64], psr[:, :, 0, :],
                                 bvr[:, :, 0, :])
            nc.vector.tensor_add(v2p[:, :, 192:256], psr[:, :, 1, :],
                                 bvr[:, :, 1, :])

        # ---- attention pieces ----
        def attn_pair(qs, j, k_hook=None, hook_first=True):
            ha, hb = 2 * j, 2 * j + 1
            q0 = qs * 512
            psu = [psum_u.tile([128, 512], F32, tag="psu",
                               name=f"psu{j}_{qs}_{i}") for i in range(2)]

            def emit_pv(k, es):
                for i, h in enumerate((ha, hb)):
                    nc.tensor.matmul(
                        psu[i][:, :], V2[k][:, h, :],
                        es[:, i * 512:(i + 1) * 512],
                        start=(k == 0), stop=(k == KB - 1))

            def emit_scores(k):
                pss = psum.tile([128, 1024], F32, tag="ps",
                                name=f"pss{j}_{qs}_{k}")
                for i, off in ((0, 0), (1, 64)):
                    nc.tensor.matmul(
                        pss[:, i * 512:(i + 1) * 512],
                        KT[j][off:off + 64, k * 128:(k + 1) * 128],
                        QT[j][off:off + 64, q0:q0 + 512],
                        start=True, stop=True)
                es = spool.tile([128, 1024], BF16, tag="es",
                                name=f"es{j}_{qs}_{k}")
                if k % 2 == 1 and k != KB - 1:
                    nc.vector.tensor_scalar(
                        es[:].bitcast(I16), pss[:],
                        expc_t[:, 0:1], expc_t[:, 1:2],
                        mybir.AluOpType.mult, mybir.AluOpType.add)
                else:
                    nc.scalar.activation(es[:], pss[:], EXPF, scale=SCALE)
                return es

            # software pipeline: scores/exp run 4-5 k-steps ahead of the
            # attnV matmuls; hook pieces (QKV m-tiles / V tiles / proj)
            # fill the tensor slack left by exp latency.
            esq = []
            next_pv = 0
            for kk in range(0, KB, 2):
                if k_hook is not None and hook_first:
                    k_hook(kk)
                for k in range(kk, min(kk + 2, KB)):
                    esq.append(emit_scores(k))
                if k_hook is not None and not hook_first:
                    k_hook(kk)
                while next_pv < kk - 4:
                    emit_pv(next_pv, esq[next_pv])
                    next_pv += 1
            while next_pv < KB:
                emit_pv(next_pv, esq[next_pv])
                next_pv += 1

            # Normalization. partition_broadcast reads its source via the
            # Q7 cpu0 read port, which only works from partition 0 — so
            # every broadcast source sits at partition 0 of its tile.
            # head b first (its denom row 0 needs no hop; frees psu
            # earlier for the DMA-lagged head a chain to catch up).
            # The [1,512] denom row is DMA-spread to [128,4] so the exact
            # reciprocal runs 4 elems/lane (a direct [1,512] reciprocal is
            # ~4us on one lane), then gathered back to partition 0 for the
            # broadcast.
            urb = rpool.tile([128, 512], F32, tag="ur2",
                             name=f"urb{j}_{qs}")
            nc.vector.tensor_copy(urb[64:128, :], psu[1][64:128, :])
            nc.scalar.copy(urb[0:1, :], psu[1][0:1, :])
            rgb = rpool.tile([128, 8], F32, tag="rg2",
                             name=f"rgb{j}_{qs}")
            nc.sync.dma_start(rgb[:, 0:4], urb[0:1, :])
            nc.vector.reciprocal(rgb[:, 4:8], rgb[:, 0:4])
            rcb = rpool.tile([128, 512], F32, tag="rb2",
                             name=f"rcb{j}_{qs}")
            nc.sync.dma_start(rcb[0:1, :], rgb[:, 4:8])
            nc.gpsimd.partition_broadcast(rcb[0:128, :], rcb[0:1, :])
            nc.vector.tensor_mul(UT[j][64:128, q0:q0 + 512],
                                 urb[64:128, :], rcb[64:128, :])
            # head a: U rows 0-63, denom row 64 (psu[0]).
            ura = rpool.tile([128, 512], F32, tag="ur",
                             name=f"ura{j}_{qs}")
            nc.scalar.copy(ura[0:65, :], psu[0][0:65, :])
            rga = rpool.tile([128, 8], F32, tag="rg",
                             name=f"rga{j}_{qs}")
            nc.sync.dma_start(rga[:, 0:4], ura[64:65, :])
            nc.vector.reciprocal(rga[:, 4:8], rga[:, 0:4])
            rca = rpool.tile([128, 512], F32, tag="rb",
                             name=f"rca{j}_{qs}")
            nc.sync.dma_start(rca[0:1, :], rga[:, 4:8])
            nc.gpsimd.partition_broadcast(rca[0:64, :], rca[0:1, :])
            nc.vector.tensor_mul(UT[j][0:64, q0:q0 + 512],
                                 ura[0:64, :], rca[0:64, :])

        def proj_qb(qb):
            pt = psum.tile([128, 1024], F32, tag="ps", name=f"pt{qb}")
            for m in range(MT):
                lhsT = UT[m][:, qb * 128:(qb + 1) * 128]
                nc.tensor.matmul(pt[:, 0:512], lhsT, wp_t[m][:, 0:512],
                                 start=(m == 0), stop=(m == MT - 1))
                nc.tensor.matmul(pt[:, 512:768], lhsT,
                                 wp_t[m][:, 512:768],
                                 start=(m == 0), stop=(m == MT - 1))
            ot = opool.tile([128, C], BF16, tag="ost", name=f"ot{qb}")
            nc.scalar.copy(ot[:, 0:512], pt[:, 0:512])
            nc.vector.tensor_copy(ot[:, 512:768], pt[:, 512:768])
            nc.sync.dma_start(out[qb * 128:(qb + 1) * 128, :], ot[:])

        # Final-chunk projection, software-pipelined: m=0 and m=2 partial
        # sums run while pair (3,1) normalizes; m=1 closes each psum tile.
        _tail_pts = {}

        def tail_open(qb):
            pt = psum.tile([128, 1024], F32, tag="ps", name=f"pt{qb}")
            _tail_pts[qb] = pt
            for m in (0, 2):
                lhsT = UT[m][:, qb * 128:(qb + 1) * 128]
                nc.tensor.matmul(pt[:, 0:512], lhsT, wp_t[m][:, 0:512],
                                 start=(m == 0), stop=False)
                nc.tensor.matmul(pt[:, 512:768], lhsT,
                                 wp_t[m][:, 512:768],
                                 start=(m == 0), stop=False)

        def tail_close(qb):
            pt = _tail_pts[qb]
            lhsT = UT[1][:, qb * 128:(qb + 1) * 128]
            nc.tensor.matmul(pt[:, 0:512], lhsT, wp_t[1][:, 0:512],
                             start=False, stop=True)
            nc.tensor.matmul(pt[:, 512:768], lhsT, wp_t[1][:, 512:768],
                             start=False, stop=True)
            ot = opool.tile([128, C], BF16, tag="ost", name=f"ot{qb}")
            nc.scalar.copy(ot[:, 0:512], pt[:, 0:512])
            nc.vector.tensor_copy(ot[:, 512:768], pt[:, 512:768])
            nc.sync.dma_start(out[qb * 128:(qb + 1) * 128, :], ot[:])

        # ---- emission schedule ----
        # Prologue: only K/Q m-tile 0 n-chunk 0. Everything else (all 16 V
        # tiles, remaining K/Q m-tiles and n-chunks, projections) drips
        # into the attention loop via hook pieces.
        pieces = []
        for t in range(3):
            pieces.append(lambda t=t: v_tile(t))
        pieces.append(lambda: qk_group(0, 1, "k"))
        pieces.append(lambda: v_tile(3))
        pieces.append(lambda: v_tile(4))
        pieces.append(lambda: qk_group(0, 2, "k"))
        pieces.append(lambda: v_tile(5))
        pieces.append(lambda: v_tile(6))
        pieces.append(lambda: qk_group(0, 3, "k"))
        for t in range(7, KB):
            pieces.append(lambda t=t: v_tile(t))
        # pair (0,1) prerequisites, then (0,2), then qs>=1 Q chunks
        pieces.append(lambda: qk_group(1, 0, "q"))
        for n in range(NQ):
            pieces.append(lambda n=n: qk_group(1, n, "k"))
        pieces.append(lambda: qk_group(2, 0, "q"))
        for n in range(NQ):
            pieces.append(lambda n=n: qk_group(2, n, "k"))
        for n in range(1, NQ):
            for m in range(MT):
                pieces.append(lambda m=m, n=n: qk_group(m, n, "q"))

        def hook(kk):
            n = 2 if len(pieces) > 4 else 1
            for _ in range(n):
                if pieces:
                    pieces.pop(0)()

        def hook00(kk):
            n = 3 if kk <= 4 else 2
            for _ in range(n):
                if pieces:
                    pieces.pop(0)()

        qk_group(0, 0, "k")
        qk_group(0, 0, "q")
        attn_pair(0, 0, k_hook=hook00, hook_first=False)
        attn_pair(0, 1, k_hook=hook)
        attn_pair(0, 2, k_hook=hook)
        for qb in range(4):
            pieces.append(lambda qb=qb: proj_qb(qb))
        for qs in range(1, NQ - 1):
            attn_pair(qs, 0, k_hook=hook)
            attn_pair(qs, 1, k_hook=hook)
            attn_pair(qs, 2, k_hook=hook)
            for qb in range(qs * 4, qs * 4 + 4):
                pieces.append(lambda qb=qb: proj_qb(qb))
        qs = NQ - 1
        attn_pair(qs, 0, k_hook=hook)
        attn_pair(qs, 2, k_hook=hook)
        attn_pair(qs, 1, k_hook=hook)
        qb0 = qs * 4
        tail_open(qb0)
        tail_open(qb0 + 1)
        tail_open(qb0 + 2)
        tail_close(qb0)
        tail_open(qb0 + 3)
        tail_close(qb0 + 1)
        tail_close(qb0 + 2)
        tail_close(qb0 + 3)

    nc.compile()
    return nc


_built = {}


def _get_nc(n_tok=2048):
    if n_tok not in _built:
        _built[n_tok] = build(n_tok)
    return _built[n_tok]


def make_in_maps(x, Wqkv, bqkv, Wproj, exp_b=EXP_B):
    B, NT, _ = x.shape
    x = np.ascontiguousarray(np.asarray(x, dtype=np.float32))
    Wqkv = np.asarray(Wqkv, dtype=np.float32)
    bqkv = np.asarray(bqkv, dtype=np.float32)
    Wproj = np.asarray(Wproj, dtype=np.float32)
    expc = np.zeros((128, 2), dtype=np.float32)
    expc[:, 0] = EXP_A
    expc[:, 1] = exp_b
    in_maps = []
    for i in range(N_CORES):
        b, g = i // 2, i % 2
        s = g * CHG
        bq = bqkv[s:s + CHG].reshape(MT, 128).T
        bk = bqkv[C + s:C + s + CHG].reshape(MT, 128).T
        in_maps.append({
            "xT": np.ascontiguousarray(x[b].T.astype(BF)),
            "wq": np.ascontiguousarray(Wqkv[:, s:s + CHG].astype(BF)),
            "wk": np.ascontiguousarray(Wqkv[:, C + s:C + s + CHG].astype(BF)),
            "wv": np.ascontiguousarray(
                Wqkv[:, 2 * C + s:2 * C + s + CHG].astype(BF)),
            "wp": np.ascontiguousarray(Wproj[s:s + CHG, :].astype(BF)),
            "bqk": np.ascontiguousarray(
                np.concatenate([bq, bk], axis=1)).astype(np.float32),
            "bv": np.ascontiguousarray(
                bqkv[2 * C + s:2 * C + s + CHG][None, :]).astype(np.float32),
            "expc": expc,
        })
    return in_maps


def gather(results, bproj, B, NT):
    parts = [np.asarray(results[i]["out"], dtype=np.float32)
             for i in range(N_CORES)]
    out = np.stack([parts[2 * b] + parts[2 * b + 1] for b in range(B)])
    return (out + np.asarray(bproj, np.float32)[None, None, :]).astype(np.float32)


def kernel(x, Wqkv, bqkv, Wproj, bproj, _trace=False, _exp_b=EXP_B):
    x = np.asarray(x)
    B, NT, _ = x.shape
    nc = _get_nc(NT)
    in_maps = make_in_maps(x, Wqkv, bqkv, Wproj, exp_b=_exp_b)
    res = run_bass_kernel_spmd(nc, in_maps, core_ids=list(range(N_CORES)),
                               trace=_trace)
    out = gather(res.results, bproj, B, NT)
    if _trace:
        return out, res
    return out


# revision 24
# speedup vs baseline: 1.5997x; 1.0449x over previous
"""Multi-head attention block (12 heads, N=2048, C=768) on 8 NeuronCores.

Sharding: core i = (batch b = i//2, head-group g = i%2). Each core computes
attention for 6 heads of one batch plus its slice of the output projection
(row-sharded Wproj); the host sums the two head-group partials per batch.

Per-core dataflow (all matmuls bf16):
  xT [768,2048] bf16 arrives host-transposed; QT/KT [384,2048] bf16 are
  column-major (head h lives at partitions (h%2)*64..+64 of tile h//2).
  V2 is token-major, 128-col stride per head (full 128-col stationary ->
  FWL). Even heads: dims at cols 0-63, ones at col 64 (denominator row
  lands at PSUM partition 64). Odd heads: ones at col 63, dims at cols
  64-127, so U_b lands directly at partitions 64-127 of PSUM and the
  normalized result writes UT[64:128] with no cross-partition DMA hop.

  Heads are processed in pairs (a=2j at PE rows 0-63, b=2j+1 at 64-127).
  Per (pair, 512-query chunk qs, key block k):
    S^T_a -> pss[:, 0:512], S^T_b -> pss[:, 512:1024]  (two matmuls in
      disjoint PE row groups sharing one PSUM tile -> concurrent)
    es = exp(S/8) in ONE instruction for both heads: ACT exp for 10 of 16
      k-blocks; DVE Schraudolph bit-trick exp for the other 6 (es_bits =
      int16(S*A + B) bitcast bf16; A,B from the `expc` input).
    U'_a += V2_a[k]^T @ es[:, 0:512], U'_b += V2_b[k]^T @ es[:, 512:1024]
      (PSUM accumulated over k; denominators via the ones column;
      software-pipelined 4-5 k-steps behind the scores)
  Normalization per head: evacuate U'+denom to SBUF, reciprocal directly
  on the [1,512] denom row, gpsimd partition-broadcast, fused multiply
  into UT -- no DMA round trips.
  out = UT^T-chunks @ Wproj_rows (bf16, PSUM-accumulated). Projection for
  query chunk qs runs as hook pieces inside the next pair; the final
  chunk's projection is software-pipelined (m=0,2 matmuls overlap the last
  pair's normalization, m=1 closes).

Startup: input DMA issue is split across three queues (Sync HWDGE: xT,
wv, wp; Scalar HWDGE: wk, wq; gpsimd SWDGE: bqk/expc/bv) because each
dma_start costs ~0.6us of issue time on its engine. All 16 V tiles and
the remaining QKV m-tiles drip into pair (0,0)'s hook.
"""

import numpy as np
import ml_dtypes
from contextlib import ExitStack

import concourse.bass as bass
import concourse.tile as tile
from concourse import bacc, mybir
from concourse.bass_utils import run_bass_kernel_spmd

N_CORES = 8
C = 768          # model dim
HG = 6           # heads per core
D = 64           # head dim
CHG = HG * D     # 384, per-group qkv width
CC = C // 128    # 6 contraction chunks
MT = CHG // 128  # 3 m-tiles for QT/KT
SCALE = 1.0 / 8.0

# Schraudolph fast-exp constants (bf16 bitcast):
#   es_bits = int16(S * EXP_A + EXP_B); bits reinterpreted as bf16
EXP_A = 128.0 * np.log2(np.e) * SCALE
EXP_B = 16248.72

F32 = mybir.dt.float32
BF16 = mybir.dt.bfloat16
I16 = mybir.dt.int16

BF = ml_dtypes.bfloat16


def build(n_tok: int = 2048):
    NT = n_tok
    KB = NT // 128           # key blocks
    NQ = NT // 512           # 512-wide query chunks
    EXPF = mybir.ActivationFunctionType.Exp

    nc = bacc.Bacc("TRN2", target_bir_lowering=False, debug=False,
                   num_devices=N_CORES)

    xT = nc.dram_tensor("xT", [C, NT], BF16, kind="ExternalInput").ap()
    wq = nc.dram_tensor("wq", [C, CHG], BF16, kind="ExternalInput").ap()
    wk = nc.dram_tensor("wk", [C, CHG], BF16, kind="ExternalInput").ap()
    wv = nc.dram_tensor("wv", [C, CHG], BF16, kind="ExternalInput").ap()
    wp = nc.dram_tensor("wp", [CHG, C], BF16, kind="ExternalInput").ap()
    bqk = nc.dram_tensor("bqk", [128, 2 * MT], F32, kind="ExternalInput").ap()
    bv = nc.dram_tensor("bv", [1, CHG], F32, kind="ExternalInput").ap()
    expc = nc.dram_tensor("expc", [128, 2], F32, kind="ExternalInput").ap()
    out = nc.dram_tensor("out", [NT, C], BF16, kind="ExternalOutput").ap()

    with tile.TileContext(nc) as tc, ExitStack() as ctx:
        wpool = ctx.enter_context(tc.tile_pool(name="w", bufs=1))
        perm = ctx.enter_context(tc.tile_pool(name="perm", bufs=1))
        # PSUM budget (8 banks): "ps" 3 x [128,1024] (6 banks) shared by
        # scores pipeline / v tiles / projection; "psu" 2 x [128,512]
        # (2 banks) holds the attnV accumulators of the in-flight pair.
        psum = ctx.enter_context(tc.tile_pool(name="ps", bufs=3, space="PSUM"))
        psum_u = ctx.enter_context(tc.tile_pool(name="psu", bufs=2,
                                                space="PSUM"))

        # ---- persistent SBUF ----
        wq_t = [wpool.tile([128, CHG], BF16, tag=f"wq{c}", name=f"wq{c}")
                for c in range(CC)]
        wk_t = [wpool.tile([128, CHG], BF16, tag=f"wk{c}", name=f"wk{c}")
                for c in range(CC)]
        wv_t = [wpool.tile([128, CHG], BF16, tag=f"wv{c}", name=f"wv{c}")
                for c in range(CC)]
        wp_t = [wpool.tile([128, C], BF16, tag=f"wp{m}", name=f"wp{m}")
                for m in range(MT)]
        bqk_t = wpool.tile([128, 2 * MT], F32, tag="bqk")
        bv_row = wpool.tile([1, CHG], F32, tag="bvr")
        bv_bc = wpool.tile([128, CHG], F32, tag="bvb")
        expc_t = wpool.tile([128, 2], F32, tag="expc")

        QT = [perm.tile([128, NT], BF16, tag=f"qt{m}", name=f"qtt{m}")
              for m in range(MT)]
        KT = [perm.tile([128, NT], BF16, tag=f"kt{m}", name=f"ktt{m}")
              for m in range(MT)]
        V2 = [perm.tile([128, HG, 128], BF16, tag=f"v2{t}", name=f"v2t{t}")
              for t in range(KB)]
        UT = [perm.tile([128, NT], BF16, tag=f"ut{m}", name=f"utt{m}")
              for m in range(MT)]

        spool = ctx.enter_context(tc.tile_pool(name="es", bufs=14))
        rpool = ctx.enter_context(tc.tile_pool(name="rb", bufs=3))
        opool = ctx.enter_context(tc.tile_pool(name="ost", bufs=3))
        xpool = ctx.enter_context(tc.tile_pool(name="xt", bufs=1))

        xt = [xpool.tile([128, NT], BF16, tag=f"x{c}", name=f"xt{c}")
              for c in range(CC)]

        # ---- input DMA, split across three issuing engines ----
        # gpsimd software DGE: the tiny side inputs
        nc.gpsimd.dma_start(bqk_t[:], bqk)
        nc.gpsimd.dma_start(expc_t[:], expc)
        nc.gpsimd.dma_start(bv_row[0:1, :], bv[0:1, :])
        # Scalar HWDGE: wk then wq (prologue order: k m-tile 0 first)
        for c in range(CC):
            nc.scalar.dma_start(wk_t[c][:], wk[c * 128:(c + 1) * 128, :])
        for c in range(CC):
            nc.scalar.dma_start(wq_t[c][:], wq[c * 128:(c + 1) * 128, :])
        # Sync HWDGE: xT (needed by everything), wv, wp
        for c in range(CC):
            nc.sync.dma_start(xt[c][:], xT[c * 128:(c + 1) * 128, :])
        for c in range(CC):
            nc.sync.dma_start(wv_t[c][:], wv[c * 128:(c + 1) * 128, :])
        for m in range(MT):
            nc.sync.dma_start(wp_t[m][:], wp[m * 128:(m + 1) * 128, :])

        nc.gpsimd.partition_broadcast(bv_bc[:], bv_row[0:1, :])
        # V2 constants: even head dims 0:64 / ones col 64; odd head ones
        # col 32 / dims 64:128 (denominator must land on a 32-aligned
        # PSUM partition). Flattened per-pair view [128, 3, 256]: ones at
        # cols {64, 160}, zeros at 65..159 and 161..191.
        bv3 = bv_bc[:, 0:3].rearrange("p (a b) -> p a b", a=MT)
        for t in range(KB):
            v2p = V2[t].rearrange("p (a b) c -> p a (b c)", b=2)
            nc.gpsimd.tensor_scalar(
                v2p[:, :, 64:65], bv3,
                0.0, 1.0, mybir.AluOpType.mult, mybir.AluOpType.add)
            nc.gpsimd.tensor_scalar(
                v2p[:, :, 128:129], bv3,
                0.0, 1.0, mybir.AluOpType.mult, mybir.AluOpType.add)
            nc.gpsimd.memset(v2p[:, :, 65:128], 0)
            nc.gpsimd.memset(v2p[:, :, 129:192], 0)

        def qk_group(m, n, which):
            wt, dst, bcol = ((wq_t, QT, m) if which == "q"
                             else (wk_t, KT, MT + m))
            ps = psum.tile([128, 512], F32, tag="ps",
                           name=f"psqk{which}{m}_{n}")
            for c in range(CC):
                nc.tensor.matmul(
                    ps[:], wt[c][:, m * 128:(m + 1) * 128],
                    xt[c][:, n * 512:(n + 1) * 512],
                    start=(c == 0), stop=(c == CC - 1))
            nc.vector.tensor_scalar_add(
                dst[m][:, n * 512:(n + 1) * 512], ps[:],
                bqk_t[:, bcol:bcol + 1])

        def v_tile(t):
            pv = psum.tile([128, 1024], F32, tag="ps", name=f"psv{t}")
            ps = pv[:, 0:CHG]
            for c in range(CC):
                nc.tensor.matmul(ps, xt[c][:, t * 128:(t + 1) * 128],
                                 wv_t[c][:],
                                 start=(c == 0), stop=(c == CC - 1))
            psr = ps.rearrange("p (a b d) -> p a b d", a=MT, b=2)
            bvr = bv_bc[:].rearrange("p (a b d) -> p a b d", a=MT, b=2)
            v2p = V2[t].rearrange("p (a b) c -> p a (b c)", b=2)
            nc.vector.tensor_add(v2p[:, :, 0:64], psr[:, :, 0, :],
                                 bvr[:, :, 0, :])
            nc.vector.tensor_add(v2p[:, :, 192:256], psr[:, :, 1, :],
                                 bvr[:, :, 1, :])

        # ---- attention pieces ----
        def attn_pair(qs, j, k_hook=None, hook_first=True, lag=2):
            ha, hb = 2 * j, 2 * j + 1
            q0 = qs * 512
            psu = [psum_u.tile([128, 512], F32, tag="psu",
                               name=f"psu{j}_{qs}_{i}") for i in range(2)]

            def emit_pv(k, es):
                for i, h in enumerate((ha, hb)):
                    nc.tensor.matmul(
                        psu[i][:, :], V2[k][:, h, :],
                        es[:, i * 512:(i + 1) * 512],
                        start=(k == 0), stop=(k == KB - 1))

            def emit_scores(k):
                pss = psum.tile([128, 1024], F32, tag="ps",
                                name=f"pss{j}_{qs}_{k}")
                for i, off in ((0, 0), (1, 64)):
                    nc.tensor.matmul(
                        pss[:, i * 512:(i + 1) * 512],
                        KT[j][off:off + 64, k * 128:(k + 1) * 128],
                        QT[j][off:off + 64, q0:q0 + 512],
                        start=True, stop=True)
                es = spool.tile([128, 1024], BF16, tag="es",
                                name=f"es{j}_{qs}_{k}")
                if k % 2 == 1 and k != KB - 1:
                    nc.vector.tensor_scalar(
                        es[:].bitcast(I16), pss[:],
                        expc_t[:, 0:1], expc_t[:, 1:2],
                        mybir.AluOpType.mult, mybir.AluOpType.add)
                else:
                    nc.scalar.activation(es[:], pss[:], EXPF, scale=SCALE)
                return es

            # software pipeline: scores/exp run 4-5 k-steps ahead of the
            # attnV matmuls; hook pieces (QKV m-tiles / V tiles / proj)
            # fill the tensor slack left by exp latency.
            esq = []
            next_pv = 0
            for kk in range(0, KB, 2):
                if k_hook is not None and hook_first:
                    k_hook(kk)
                for k in range(kk, min(kk + 2, KB)):
                    esq.append(emit_scores(k))
                if k_hook is not None and not hook_first:
                    k_hook(kk)
                while next_pv < kk - lag:
                    emit_pv(next_pv, esq[next_pv])
                    next_pv += 1
            while next_pv < KB:
                emit_pv(next_pv, esq[next_pv])
                next_pv += 1

            # Normalization. partition_broadcast reads its source via the
            # Q7 cpu0 read port, which only works from partition 0 — so
            # every broadcast source sits at partition 0 of its tile.
            # head b first (its denom row 0 needs no hop; frees psu
            # earlier for the DMA-lagged head a chain to catch up).
            # The [1,512] denom row is DMA-spread to [128,4] so the exact
            # reciprocal runs 4 elems/lane (a direct [1,512] reciprocal is
            # ~4us on one lane), then gathered back to partition 0 for the
            # broadcast.
            urb = rpool.tile([128, 512], F32, tag="ur2",
                             name=f"urb{j}_{qs}")
            nc.vector.tensor_copy(urb[64:128, :], psu[1][64:128, :])
            nc.scalar.copy(urb[0:1, :], psu[1][0:1, :])
            rgb = rpool.tile([128, 8], F32, tag="rg2",
                             name=f"rgb{j}_{qs}")
            nc.sync.dma_start(rgb[:, 0:4], urb[0:1, :])
            nc.vector.reciprocal(rgb[:, 4:8], rgb[:, 0:4])
            rcb = rpool.tile([128, 512], F32, tag="rb2",
                             name=f"rcb{j}_{qs}")
            nc.sync.dma_start(rcb[0:1, :], rgb[:, 4:8])
            nc.gpsimd.partition_broadcast(rcb[0:128, :], rcb[0:1, :])
            nc.vector.tensor_mul(UT[j][64:128, q0:q0 + 512],
                                 urb[64:128, :], rcb[64:128, :])
            # head a: U rows 0-63, denom row 64 (psu[0]).
            ura = rpool.tile([128, 512], F32, tag="ur",
                             name=f"ura{j}_{qs}")
            nc.scalar.copy(ura[0:65, :], psu[0][0:65, :])
            rga = rpool.tile([128, 8], F32, tag="rg",
                             name=f"rga{j}_{qs}")
            nc.sync.dma_start(rga[:, 0:4], ura[64:65, :])
            nc.vector.reciprocal(rga[:, 4:8], rga[:, 0:4])
            rca = rpool.tile([128, 512], F32, tag="rb",
                             name=f"rca{j}_{qs}")
            nc.sync.dma_start(rca[0:1, :], rga[:, 4:8])
            nc.gpsimd.partition_broadcast(rca[0:64, :], rca[0:1, :])
            nc.vector.tensor_mul(UT[j][0:64, q0:q0 + 512],
                                 ura[0:64, :], rca[0:64, :])

        def proj_qb(qb):
            pt = psum.tile([128, 1024], F32, tag="ps", name=f"pt{qb}")
            for m in range(MT):
                lhsT = UT[m][:, qb * 128:(qb + 1) * 128]
                nc.tensor.matmul(pt[:, 0:512], lhsT, wp_t[m][:, 0:512],
                                 start=(m == 0), stop=(m == MT - 1))
                nc.tensor.matmul(pt[:, 512:768], lhsT,
                                 wp_t[m][:, 512:768],
                                 start=(m == 0), stop=(m == MT - 1))
            ot = opool.tile([128, C], BF16, tag="ost", name=f"ot{qb}")
            nc.scalar.copy(ot[:, 0:512], pt[:, 0:512])
            nc.vector.tensor_copy(ot[:, 512:768], pt[:, 512:768])
            nc.sync.dma_start(out[qb * 128:(qb + 1) * 128, :], ot[:])

        # Final-chunk projection, software-pipelined: m=0 and m=2 partial
        # sums run while pair (3,1) normalizes; m=1 closes each psum tile.
        _tail_pts = {}

        def tail_open(qb):
            pt = psum.tile([128, 1024], F32, tag="ps", name=f"pt{qb}")
            _tail_pts[qb] = pt
            for m in (0, 2):
                lhsT = UT[m][:, qb * 128:(qb + 1) * 128]
                nc.tensor.matmul(pt[:, 0:512], lhsT, wp_t[m][:, 0:512],
                                 start=(m == 0), stop=False)
                nc.tensor.matmul(pt[:, 512:768], lhsT,
                                 wp_t[m][:, 512:768],
                                 start=(m == 0), stop=False)

        def tail_close(qb):
            pt = _tail_pts[qb]
            lhsT = UT[1][:, qb * 128:(qb + 1) * 128]
            nc.tensor.matmul(pt[:, 0:512], lhsT, wp_t[1][:, 0:512],
                             start=False, stop=True)
            nc.tensor.matmul(pt[:, 512:768], lhsT, wp_t[1][:, 512:768],
                             start=False, stop=True)
            ot = opool.tile([128, C], BF16, tag="ost", name=f"ot{qb}")
            nc.scalar.copy(ot[:, 0:512], pt[:, 0:512])
            nc.vector.tensor_copy(ot[:, 512:768], pt[:, 512:768])
            nc.sync.dma_start(out[qb * 128:(qb + 1) * 128, :], ot[:])

        # ---- emission schedule ----
        # Prologue: only K/Q m-tile 0 n-chunk 0. Everything else (all 16 V
        # tiles, remaining K/Q m-tiles and n-chunks, projections) drips
        # into the attention loop via hook pieces.
        pieces = []
        for t in range(3):
            pieces.append(lambda t=t: v_tile(t))
        pieces.append(lambda: qk_group(0, 1, "k"))
        pieces.append(lambda: v_tile(3))
        pieces.append(lambda: v_tile(4))
        pieces.append(lambda: qk_group(0, 2, "k"))
        pieces.append(lambda: v_tile(5))
        pieces.append(lambda: v_tile(6))
        pieces.append(lambda: qk_group(0, 3, "k"))
        for t in range(7, KB):
            pieces.append(lambda t=t: v_tile(t))
        # pair (0,1) prerequisites, then (0,2), then qs>=1 Q chunks
        pieces.append(lambda: qk_group(1, 0, "q"))
        for n in range(NQ):
            pieces.append(lambda n=n: qk_group(1, n, "k"))
        pieces.append(lambda: qk_group(2, 0, "q"))
        for n in range(NQ):
            pieces.append(lambda n=n: qk_group(2, n, "k"))
        for n in range(1, NQ):
            for m in range(MT):
                pieces.append(lambda m=m, n=n: qk_group(m, n, "q"))

        def hook(kk):
            n = 2 if len(pieces) > 4 else 1
            for _ in range(n):
                if pieces:
                    pieces.pop(0)()

        def hook00(kk):
            n = 3 if kk <= 4 else 2
            for _ in range(n):
                if pieces:
                    pieces.pop(0)()

        qk_group(0, 0, "k")
        qk_group(0, 0, "q")
        attn_pair(0, 0, k_hook=hook00, hook_first=False, lag=4)
        attn_pair(0, 1, k_hook=hook)
        attn_pair(0, 2, k_hook=hook)
        for qs in range(1, NQ - 1):
            attn_pair(qs, 0, k_hook=hook)
            for qb in range((qs - 1) * 4, (qs - 1) * 4 + 4):
                proj_qb(qb)
            attn_pair(qs, 1, k_hook=hook)
            attn_pair(qs, 2, k_hook=hook)
        qs = NQ - 1
        attn_pair(qs, 0, k_hook=hook)
        for qb in range((qs - 1) * 4, (qs - 1) * 4 + 4):
            proj_qb(qb)
        attn_pair(qs, 2, k_hook=hook)
        attn_pair(qs, 1, k_hook=hook)
        qb0 = qs * 4
        tail_open(qb0)
        tail_open(qb0 + 1)
        tail_open(qb0 + 2)
        tail_close(qb0)
        tail_open(qb0 + 3)
        tail_close(qb0 + 1)
        tail_close(qb0 + 2)
        tail_close(qb0 + 3)

    nc.compile()
    return nc


_built = {}


def _get_nc(n_tok=2048):
    if n_tok not in _built:
        _built[n_tok] = build(n_tok)
    return _built[n_tok]


def make_in_maps(x, Wqkv, bqkv, Wproj, exp_b=EXP_B):
    B, NT, _ = x.shape
    x = np.ascontiguousarray(np.asarray(x, dtype=np.float32))
    Wqkv = np.asarray(Wqkv, dtype=np.float32)
    bqkv = np.asarray(bqkv, dtype=np.float32)
    Wproj = np.asarray(Wproj, dtype=np.float32)
    expc = np.zeros((128, 2), dtype=np.float32)
    expc[:, 0] = EXP_A
    expc[:, 1] = exp_b
    in_maps = []
    for i in range(N_CORES):
        b, g = i // 2, i % 2
        s = g * CHG
        bq = bqkv[s:s + CHG].reshape(MT, 128).T
        bk = bqkv[C + s:C + s + CHG].reshape(MT, 128).T
        in_maps.append({
            "xT": np.ascontiguousarray(x[b].T.astype(BF)),
            "wq": np.ascontiguousarray(Wqkv[:, s:s + CHG].astype(BF)),
            "wk": np.ascontiguousarray(Wqkv[:, C + s:C + s + CHG].astype(BF)),
            "wv": np.ascontiguousarray(
                Wqkv[:, 2 * C + s:2 * C + s + CHG].astype(BF)),
            "wp": np.ascontiguousarray(Wproj[s:s + CHG, :].astype(BF)),
            "bqk": np.ascontiguousarray(
                np.concatenate([bq, bk], axis=1)).astype(np.float32),
            "bv": np.ascontiguousarray(
                bqkv[2 * C + s:2 * C + s + CHG][None, :]).astype(np.float32),
            "expc": expc,
        })
    return in_maps


def gather(results, bproj, B, NT):
    parts = [np.asarray(results[i]["out"], dtype=np.float32)
             for i in range(N_CORES)]
    out = np.stack([parts[2 * b] + parts[2 * b + 1] for b in range(B)])
    return (out + np.asarray(bproj, np.float32)[None, None, :]).astype(np.float32)


def kernel(x, Wqkv, bqkv, Wproj, bproj, _trace=False, _exp_b=EXP_B):
    x = np.asarray(x)
    B, NT, _ = x.shape
    nc = _get_nc(NT)
    in_maps = make_in_maps(x, Wqkv, bqkv, Wproj, exp_b=_exp_b)
    res = run_bass_kernel_spmd(nc, in_maps, core_ids=list(range(N_CORES)),
                               trace=_trace)
    out = gather(res.results, bproj, B, NT)
    if _trace:
        return out, res
    return out
